# revision 8
# baseline (speedup 1.0000x reference)
"""Domain discrepancy (MMD-style) loss kernel for 8 Trainium2 NeuronCores.

reference computes, for S, T in R^{4096 x 2048}:
    k(x, y) = exp(-||x - y||^2 / d^2),   d = 2048
    out = mean(Kss) + mean(Ktt) - 2 * mean(Kst)        (float32 scalar)

Strategy
--------
All kernel arguments z = -||x-y||^2/d^2 lie within ~1.2e-3 of z0 = -2/d, so
k = exp(z0) * e^w with w = z - z0, |w| <~ 1e-3.  A 2nd-order Taylor expansion
of e^w is exact to ~1e-16 per element, which turns the three kernel-matrix
means into
    sum_ij k = c * (N*M + Sum(w) + Sum(w^2)/2),   c = exp(z0)
with w_ij = 2*<x_i, y_j>/d^2 + hb_i + hc_j, hb_i = (d - ||x_i||^2)/d^2.
Sum(w) and the bias cross-terms of Sum(w^2) collapse to O(N*D) analytic sums
(host, float64); only Sum_ij <x_i,y_j>^2 needs the pairwise matrices.

All three Gram-squared sums live inside the symmetric 8192x8192 pairwise
matrix of Z = [S; T]: only its upper-triangle 512x512 blocks are computed —
136 block-GEMMs instead of the 192 a direct 3-matrix pass needs (-29% PE
work).  Each core gets 17 blocks (row-pair P=c with P=15-c balances the
triangle exactly).  GEMMs run in fp8 (e4m3) DoubleRow; each PSUM tile is
reduced by one VectorE bn_stats op (count/mean/M2 -> Sum(ps), Sum(ps^2)).
The host routes each block's sum to xx/yy/xy (P,Q<8 -> xx, P,Q>=8 -> yy,
mixed -> xy, off-diagonal blocks doubled) and assembles the three means in
float64.

The final means are combined in float32 exactly like the reference
(xx + yy - 2*xy on fp32-rounded means), reproducing its arithmetic.
"""

import numpy as np
import ml_dtypes
from contextlib import ExitStack

import concourse.bass as bass
import concourse.tile as tile
from concourse import bacc, mybir
from concourse import bass_utils

N, D = 4096, 2048
NCORES = 8
NB = 16                    # 512-row blocks of Z (8192 rows)
TPC = 17                   # triangle blocks per core
IC = 4                     # 128-row i-chunks per block
KB = D // 128              # 16 contraction chunks of 128
KK = KB // 2               # 8 DoubleRow steps of 256
SCALE = float(2.0 / (D * D))
F32 = mybir.dt.float32
FP8 = mybir.dt.float8e4

_compiled = {}


def blocks_for_core(c):
    out = [(c, q) for q in range(c, NB)]
    out += [(NB - 1 - c, q) for q in range(NB - 1 - c, NB)]
    return out


def _build():
    nc = bacc.Bacc("TRN2", target_bir_lowering=False, debug=False,
                   num_devices=NCORES)

    sta_all = nc.dram_tensor("sta_all", [TPC, 128, KB * 512], FP8, kind="ExternalInput")
    mov_all = nc.dram_tensor("mov_all", [TPC, 128, KB * 512], FP8, kind="ExternalInput")
    out = nc.dram_tensor("out", [128, TPC * IC * 6], F32, kind="ExternalOutput")

    with tile.TileContext(nc) as tc, ExitStack() as ctx:
        const_pool = ctx.enter_context(tc.tile_pool(name="const", bufs=1))
        slab_pool = ctx.enter_context(tc.tile_pool(name="slabs", bufs=4))
        psum_pool = ctx.enter_context(tc.tile_pool(name="psum", bufs=8, space="PSUM"))

        out_sb = const_pool.tile([128, TPC * IC * 6], F32, tag="out_sb")
        sta_ap = sta_all.ap()
        mov_ap = mov_all.ap()

        for t in range(TPC):
            sta = slab_pool.tile([128, KB * 512], FP8, tag="sta")
            mov = slab_pool.tile([128, KB * 512], FP8, tag="mov")
            if t == 0:
                # k-chunked loads so the first matmuls start after ~150KB
                # instead of the full 2MB
                for kk in range(KK):
                    cs = slice(kk * 2 * 512, (kk + 1) * 2 * 512)
                    nc.sync.dma_start(sta[:, cs], sta_ap[t][:, cs])
                    nc.sync.dma_start(mov[:, cs], mov_ap[t][:, cs])
            else:
                nc.sync.dma_start(sta[:], sta_ap[t])
                nc.sync.dma_start(mov[:], mov_ap[t])
            sta3 = sta[:].rearrange("p (k i) -> p k i", k=KB)
            mov3 = mov[:].rearrange("p (k j) -> p k j", k=KB)
            for ic in range(IC):
                ps = psum_pool.tile([128, 512], F32, tag="ps", name=f"ps_{t}_{ic}")
                for kk in range(KK):
                    nc.tensor.matmul(
                        ps[:],
                        sta3[:, 2 * kk:2 * kk + 2, ic * 128:(ic + 1) * 128],
                        mov3[:, 2 * kk:2 * kk + 2, :],
                        start=(kk == 0), stop=(kk == KK - 1),
                        perf_mode=mybir.MatmulPerfMode.DoubleRow,
                    )
                col = (t * IC + ic) * 6
                nc.vector.bn_stats(out_sb[:, col:col + 6], ps[:])
        nc.sync.dma_start(out.ap(), out_sb[:])

    nc.compile()
    return nc


def _get_nc():
    if "nc" not in _compiled:
        _compiled["nc"] = _build()
    return _compiled["nc"]


def _prep_inputs(S, T):
    """Host-side shard/layout prep (float32 -> fp8 e4m3, transposed tilings)."""
    Sb = S.astype(ml_dtypes.float8_e4m3)
    Tb = T.astype(ml_dtypes.float8_e4m3)
    Zq = np.vstack([Sb, Tb])

    def rows(P):
        # r[p, k*512+i] = Z[P*512+i, 128k+p]
        blk = Zq[P * 512:(P + 1) * 512]
        return np.ascontiguousarray(
            blk.reshape(512, KB, 128).transpose(2, 1, 0)
        ).reshape(128, KB * 512)

    tiles = [rows(P) for P in range(NB)]
    in_maps = []
    for c in range(NCORES):
        blks = blocks_for_core(c)
        in_maps.append({
            "sta_all": np.stack([tiles[P] for P, _ in blks]),
            "mov_all": np.stack([tiles[Q] for _, Q in blks]),
        })
    return in_maps, Sb, Tb


def _combine(per_core_outs, S, T, Sb, Tb):
    """Host float64 combination of device partial sums -> the three means."""
    S64, T64 = S.astype(np.float64), T.astype(np.float64)
    Sq64, Tq64 = Sb.astype(np.float64), Tb.astype(np.float64)
    x2 = (S64 ** 2).sum(1)
    y2 = (T64 ** 2).sum(1)
    hbS = (D - x2) / (D * D)
    hbT = (D - y2) / (D * D)
    sSq = Sq64.sum(0)
    sTq = Tq64.sum(0)

    # decode bn_stats -> per-block Sum(ps^2), route to xx/yy/xy
    Bsum = np.zeros(3)
    for c, o in enumerate(per_core_outs):
        o = o.astype(np.float64).reshape(128, TPC * IC, 6)
        sq = (o[:, :, 2] + 256.0 * o[:, :, 1] ** 2
              + o[:, :, 5] + 256.0 * o[:, :, 4] ** 2)
        sq = sq.sum(axis=0).reshape(TPC, IC).sum(axis=1)
        for t, (P, Q) in enumerate(blocks_for_core(c)):
            if P < 8 and Q < 8:
                Bsum[0] += sq[t] * (1.0 if P == Q else 2.0)
            elif P >= 8 and Q >= 8:
                Bsum[1] += sq[t] * (1.0 if P == Q else 2.0)
            else:
                Bsum[2] += sq[t]

    cfg = [
        (hbS, hbS, Sq64, Sq64, sSq, sSq),   # xx
        (hbT, hbT, Tq64, Tq64, sTq, sTq),   # yy
        (hbS, hbT, Sq64, Tq64, sSq, sTq),   # xy: i-side S, j-side T
    ]
    c0 = np.exp(-2.0 / D)
    s = SCALE
    means = []
    for mat, (hb, hc, U, V, sU, sV) in enumerate(cfg):
        Sw = s * (sU @ sV) + N * hb.sum() + N * hc.sum()
        Sw2 = (s * s * Bsum[mat] + N * (hb ** 2).sum() + N * (hc ** 2).sum()
               + 2.0 * hb.sum() * hc.sum()
               + 2.0 * s * (hb @ (U @ sV) + hc @ (V @ sU)))
        means.append(c0 * (1.0 + (Sw + 0.5 * Sw2) / (float(N) * N)))
    return means


def kernel(source_features, target_features):
    S = np.asarray(source_features, dtype=np.float32)
    T = np.asarray(target_features, dtype=np.float32)

    nc = _get_nc()
    in_maps, Sb, Tb = _prep_inputs(S, T)
    import os
    trace = bool(int(os.environ.get("BASS_KERNEL_TRACE", "0")))
    res = bass_utils.run_bass_kernel_spmd(
        nc, in_maps, core_ids=list(range(NCORES)), trace=trace)
    _compiled["last_results"] = res
    per_core = [np.asarray(r["out"], np.float32) for r in res.results]

    means = _combine(per_core, S, T, Sb, Tb)
    f = np.float32
    xx, yy, xy = (f(m) for m in means)
    val = f(f(xx + yy) - f(2.0) * xy)
    return np.array(val, dtype=np.float32)


# revision 10
# speedup vs baseline: 1.0611x; 1.0611x over previous
"""Domain discrepancy (MMD-style) loss kernel for 8 Trainium2 NeuronCores.

reference computes, for S, T in R^{4096 x 2048}:
    k(x, y) = exp(-||x - y||^2 / d^2),   d = 2048
    out = mean(Kss) + mean(Ktt) - 2 * mean(Kst)        (float32 scalar)

Strategy
--------
All kernel arguments z = -||x-y||^2/d^2 lie within ~1.2e-3 of z0 = -2/d, so
k = exp(z0) * e^w with w = z - z0, |w| <~ 1e-3.  A 2nd-order Taylor expansion
of e^w is exact to ~1e-16 per element, which turns the three kernel-matrix
means into
    sum_ij k = c * (N*M + Sum(w) + Sum(w^2)/2),   c = exp(z0)
with w_ij = 2*<x_i, y_j>/d^2 + hb_i + hc_j, hb_i = (d - ||x_i||^2)/d^2.
Sum(w) and the bias cross-terms of Sum(w^2) collapse to O(N*D) analytic sums
(host, float64); only Sum_ij <x_i,y_j>^2 needs the pairwise matrices.

All three Gram-squared sums live inside the symmetric 8192x8192 pairwise
matrix of Z = [S; T]: only its upper-triangle 512x512 blocks are computed —
136 block-GEMMs instead of the 192 a direct 3-matrix pass needs (-29% PE
work).  Each core gets 17 blocks (row-pair P=c with P=15-c balances the
triangle exactly).  GEMMs run in fp8 (e4m3) DoubleRow; each PSUM tile is
reduced by one VectorE bn_stats op (count/mean/M2 -> Sum(ps), Sum(ps^2)).
The host routes each block's sum to xx/yy/xy (P,Q<8 -> xx, P,Q>=8 -> yy,
mixed -> xy, off-diagonal blocks doubled) and assembles the three means in
float64.

The final means are combined in float32 exactly like the reference
(xx + yy - 2*xy on fp32-rounded means), reproducing its arithmetic.
"""

import numpy as np
import ml_dtypes
from contextlib import ExitStack

import concourse.bass as bass
import concourse.tile as tile
from concourse import bacc, mybir
from concourse import bass_utils

N, D = 4096, 2048
NCORES = 8
NB = 16                    # 512-row blocks of Z (8192 rows)
TPC = 17                   # triangle blocks per core
IC = 4                     # 128-row i-chunks per block
KB = D // 128              # 16 contraction chunks of 128
KK = KB // 2               # 8 DoubleRow steps of 256
SCALE = float(2.0 / (D * D))
F32 = mybir.dt.float32
FP8 = mybir.dt.float8e4

_compiled = {}


def blocks_for_core(c):
    out = [(c, q) for q in range(c, NB)]
    out += [(NB - 1 - c, q) for q in range(NB - 1 - c, NB)]
    return out


def _build():
    nc = bacc.Bacc("TRN2", target_bir_lowering=False, debug=False,
                   num_devices=NCORES)

    sta_all = nc.dram_tensor("sta_all", [TPC, 128, KB * 512], FP8, kind="ExternalInput")
    mov_all = nc.dram_tensor("mov_all", [TPC, 128, KB * 512], FP8, kind="ExternalInput")
    out = nc.dram_tensor("out", [128, TPC * IC * 6], F32, kind="ExternalOutput")

    with tile.TileContext(nc) as tc, ExitStack() as ctx:
        const_pool = ctx.enter_context(tc.tile_pool(name="const", bufs=1))
        slab_pool = ctx.enter_context(tc.tile_pool(name="slabs", bufs=6))
        psum_pool = ctx.enter_context(tc.tile_pool(name="psum", bufs=8, space="PSUM"))

        out_sb = const_pool.tile([128, TPC * IC * 6], F32, tag="out_sb")
        sta_ap = sta_all.ap()
        mov_ap = mov_all.ap()

        for t in range(TPC):
            sta = slab_pool.tile([128, KB * 512], FP8, tag="sta")
            nc.sync.dma_start(sta[:], sta_ap[t])
            mov = slab_pool.tile([128, KB * 512], FP8, tag="mov")
            nc.sync.dma_start(mov[:], mov_ap[t])
            sta3 = sta[:].rearrange("p (k i) -> p k i", k=KB)
            mov3 = mov[:].rearrange("p (k j) -> p k j", k=KB)
            for ic in range(IC):
                ps = psum_pool.tile([128, 512], F32, tag="ps", name=f"ps_{t}_{ic}")
                for kk in range(KK):
                    nc.tensor.matmul(
                        ps[:],
                        sta3[:, 2 * kk:2 * kk + 2, ic * 128:(ic + 1) * 128],
                        mov3[:, 2 * kk:2 * kk + 2, :],
                        start=(kk == 0), stop=(kk == KK - 1),
                        perf_mode=mybir.MatmulPerfMode.DoubleRow,
                    )
                col = (t * IC + ic) * 6
                nc.vector.bn_stats(out_sb[:, col:col + 6], ps[:])
        nc.sync.dma_start(out.ap(), out_sb[:])

    nc.compile()
    return nc


def _get_nc():
    if "nc" not in _compiled:
        _compiled["nc"] = _build()
    return _compiled["nc"]


def _prep_inputs(S, T):
    """Host-side shard/layout prep (float32 -> fp8 e4m3, transposed tilings)."""
    Sb = S.astype(ml_dtypes.float8_e4m3)
    Tb = T.astype(ml_dtypes.float8_e4m3)
    Zq = np.vstack([Sb, Tb])

    def rows(P):
        # r[p, k*512+i] = Z[P*512+i, 128k+p]
        blk = Zq[P * 512:(P + 1) * 512]
        return np.ascontiguousarray(
            blk.reshape(512, KB, 128).transpose(2, 1, 0)
        ).reshape(128, KB * 512)

    tiles = [rows(P) for P in range(NB)]
    in_maps = []
    for c in range(NCORES):
        blks = blocks_for_core(c)
        in_maps.append({
            "sta_all": np.stack([tiles[P] for P, _ in blks]),
            "mov_all": np.stack([tiles[Q] for _, Q in blks]),
        })
    return in_maps, Sb, Tb


def _combine(per_core_outs, S, T, Sb, Tb):
    """Host float64 combination of device partial sums -> the three means."""
    S64, T64 = S.astype(np.float64), T.astype(np.float64)
    Sq64, Tq64 = Sb.astype(np.float64), Tb.astype(np.float64)
    x2 = (S64 ** 2).sum(1)
    y2 = (T64 ** 2).sum(1)
    hbS = (D - x2) / (D * D)
    hbT = (D - y2) / (D * D)
    sSq = Sq64.sum(0)
    sTq = Tq64.sum(0)

    # decode bn_stats -> per-block Sum(ps^2), route to xx/yy/xy
    Bsum = np.zeros(3)
    for c, o in enumerate(per_core_outs):
        o = o.astype(np.float64).reshape(128, TPC * IC, 6)
        sq = (o[:, :, 2] + 256.0 * o[:, :, 1] ** 2
              + o[:, :, 5] + 256.0 * o[:, :, 4] ** 2)
        sq = sq.sum(axis=0).reshape(TPC, IC).sum(axis=1)
        for t, (P, Q) in enumerate(blocks_for_core(c)):
            if P < 8 and Q < 8:
                Bsum[0] += sq[t] * (1.0 if P == Q else 2.0)
            elif P >= 8 and Q >= 8:
                Bsum[1] += sq[t] * (1.0 if P == Q else 2.0)
            else:
                Bsum[2] += sq[t]

    cfg = [
        (hbS, hbS, Sq64, Sq64, sSq, sSq),   # xx
        (hbT, hbT, Tq64, Tq64, sTq, sTq),   # yy
        (hbS, hbT, Sq64, Tq64, sSq, sTq),   # xy: i-side S, j-side T
    ]
    c0 = np.exp(-2.0 / D)
    s = SCALE
    means = []
    for mat, (hb, hc, U, V, sU, sV) in enumerate(cfg):
        Sw = s * (sU @ sV) + N * hb.sum() + N * hc.sum()
        Sw2 = (s * s * Bsum[mat] + N * (hb ** 2).sum() + N * (hc ** 2).sum()
               + 2.0 * hb.sum() * hc.sum()
               + 2.0 * s * (hb @ (U @ sV) + hc @ (V @ sU)))
        means.append(c0 * (1.0 + (Sw + 0.5 * Sw2) / (float(N) * N)))
    return means


def kernel(source_features, target_features):
    S = np.asarray(source_features, dtype=np.float32)
    T = np.asarray(target_features, dtype=np.float32)

    nc = _get_nc()
    in_maps, Sb, Tb = _prep_inputs(S, T)
    import os
    trace = bool(int(os.environ.get("BASS_KERNEL_TRACE", "0")))
    res = bass_utils.run_bass_kernel_spmd(
        nc, in_maps, core_ids=list(range(NCORES)), trace=trace)
    _compiled["last_results"] = res
    per_core = [np.asarray(r["out"], np.float32) for r in res.results]

    means = _combine(per_core, S, T, Sb, Tb)
    f = np.float32
    xx, yy, xy = (f(m) for m in means)
    val = f(f(xx + yy) - f(2.0) * xy)
    return np.array(val, dtype=np.float32)


# revision 11
# speedup vs baseline: 1.0715x; 1.0099x over previous
"""Domain discrepancy (MMD-style) loss kernel for 8 Trainium2 NeuronCores.

reference computes, for S, T in R^{4096 x 2048}:
    k(x, y) = exp(-||x - y||^2 / d^2),   d = 2048
    out = mean(Kss) + mean(Ktt) - 2 * mean(Kst)        (float32 scalar)

Strategy
--------
All kernel arguments z = -||x-y||^2/d^2 lie within ~1.2e-3 of z0 = -2/d, so
k = exp(z0) * e^w with w = z - z0, |w| <~ 1e-3.  A 2nd-order Taylor expansion
of e^w is exact to ~1e-16 per element, which turns the three kernel-matrix
means into
    sum_ij k = c * (N*M + Sum(w) + Sum(w^2)/2),   c = exp(z0)
with w_ij = 2*<x_i, y_j>/d^2 + hb_i + hc_j, hb_i = (d - ||x_i||^2)/d^2.
Sum(w) and the bias cross-terms of Sum(w^2) collapse to O(N*D) analytic sums
(host, float64); only Sum_ij <x_i,y_j>^2 needs the pairwise matrices.

All three Gram-squared sums live inside the symmetric 8192x8192 pairwise
matrix of Z = [S; T]: only its upper-triangle 512x512 blocks are computed —
136 block-GEMMs instead of the 192 a direct 3-matrix pass needs (-29% PE
work).  Each core gets 17 blocks (row-pair P=c with P=15-c balances the
triangle exactly).  GEMMs run in fp8 (e4m3) DoubleRow; each PSUM tile is
reduced by one VectorE bn_stats op (count/mean/M2 -> Sum(ps), Sum(ps^2)).
The host routes each block's sum to xx/yy/xy (P,Q<8 -> xx, P,Q>=8 -> yy,
mixed -> xy, off-diagonal blocks doubled) and assembles the three means in
float64.

The final means are combined in float32 exactly like the reference
(xx + yy - 2*xy on fp32-rounded means), reproducing its arithmetic.
"""

import numpy as np
import ml_dtypes
from contextlib import ExitStack

import concourse.bass as bass
import concourse.tile as tile
from concourse import bacc, mybir
from concourse import bass_utils

N, D = 4096, 2048
NCORES = 8
NB = 16                    # 512-row blocks of Z (8192 rows)
TPC = 17                   # triangle blocks per core
IC = 4                     # 128-row i-chunks per block
KB = D // 128              # 16 contraction chunks of 128
KK = KB // 2               # 8 DoubleRow steps of 256
SCALE = float(2.0 / (D * D))
F32 = mybir.dt.float32
FP8 = mybir.dt.float8e4

_compiled = {}


def blocks_for_core(c):
    out = [(c, q) for q in range(c, NB)]
    out += [(NB - 1 - c, q) for q in range(NB - 1 - c, NB)]
    return out


def _build():
    nc = bacc.Bacc("TRN2", target_bir_lowering=False, debug=False,
                   num_devices=NCORES)

    sta_all = nc.dram_tensor("sta_all", [TPC, 128, KB * 512], FP8, kind="ExternalInput")
    mov_all = nc.dram_tensor("mov_all", [TPC, 128, KB * 512], FP8, kind="ExternalInput")
    out = nc.dram_tensor("out", [128, TPC * IC * 6], F32, kind="ExternalOutput")

    with tile.TileContext(nc) as tc, ExitStack() as ctx:
        const_pool = ctx.enter_context(tc.tile_pool(name="const", bufs=1))
        slab_pool = ctx.enter_context(tc.tile_pool(name="slabs", bufs=6))
        psum_pool = ctx.enter_context(tc.tile_pool(name="psum", bufs=8, space="PSUM"))

        out_sb = const_pool.tile([128, TPC * IC * 6], F32, tag="out_sb")
        sta_ap = sta_all.ap()
        mov_ap = mov_all.ap()

        H = KB * 512 // 2
        for t in range(TPC):
            sta = slab_pool.tile([128, KB * 512], FP8, tag="sta")
            mov = slab_pool.tile([128, KB * 512], FP8, tag="mov")
            if t == 0:
                # halved first loads so the first matmuls start ~3us earlier
                nc.sync.dma_start(sta[:, :H], sta_ap[t][:, :H])
                nc.sync.dma_start(mov[:, :H], mov_ap[t][:, :H])
                nc.sync.dma_start(sta[:, H:], sta_ap[t][:, H:])
                nc.sync.dma_start(mov[:, H:], mov_ap[t][:, H:])
            else:
                nc.sync.dma_start(sta[:], sta_ap[t])
                nc.sync.dma_start(mov[:], mov_ap[t])
            sta3 = sta[:].rearrange("p (k i) -> p k i", k=KB)
            mov3 = mov[:].rearrange("p (k j) -> p k j", k=KB)
            for ic in range(IC):
                ps = psum_pool.tile([128, 512], F32, tag="ps", name=f"ps_{t}_{ic}")
                for kk in range(KK):
                    nc.tensor.matmul(
                        ps[:],
                        sta3[:, 2 * kk:2 * kk + 2, ic * 128:(ic + 1) * 128],
                        mov3[:, 2 * kk:2 * kk + 2, :],
                        start=(kk == 0), stop=(kk == KK - 1),
                        perf_mode=mybir.MatmulPerfMode.DoubleRow,
                    )
                col = (t * IC + ic) * 6
                nc.vector.bn_stats(out_sb[:, col:col + 6], ps[:])
        nc.sync.dma_start(out.ap(), out_sb[:])

    nc.compile()
    return nc


def _get_nc():
    if "nc" not in _compiled:
        _compiled["nc"] = _build()
    return _compiled["nc"]


def _prep_inputs(S, T):
    """Host-side shard/layout prep (float32 -> fp8 e4m3, transposed tilings)."""
    Sb = S.astype(ml_dtypes.float8_e4m3)
    Tb = T.astype(ml_dtypes.float8_e4m3)
    Zq = np.vstack([Sb, Tb])

    def rows(P):
        # r[p, k*512+i] = Z[P*512+i, 128k+p]
        blk = Zq[P * 512:(P + 1) * 512]
        return np.ascontiguousarray(
            blk.reshape(512, KB, 128).transpose(2, 1, 0)
        ).reshape(128, KB * 512)

    tiles = [rows(P) for P in range(NB)]
    in_maps = []
    for c in range(NCORES):
        blks = blocks_for_core(c)
        in_maps.append({
            "sta_all": np.stack([tiles[P] for P, _ in blks]),
            "mov_all": np.stack([tiles[Q] for _, Q in blks]),
        })
    return in_maps, Sb, Tb


def _combine(per_core_outs, S, T, Sb, Tb):
    """Host float64 combination of device partial sums -> the three means."""
    S64, T64 = S.astype(np.float64), T.astype(np.float64)
    Sq64, Tq64 = Sb.astype(np.float64), Tb.astype(np.float64)
    x2 = (S64 ** 2).sum(1)
    y2 = (T64 ** 2).sum(1)
    hbS = (D - x2) / (D * D)
    hbT = (D - y2) / (D * D)
    sSq = Sq64.sum(0)
    sTq = Tq64.sum(0)

    # decode bn_stats -> per-block Sum(ps^2), route to xx/yy/xy
    Bsum = np.zeros(3)
    for c, o in enumerate(per_core_outs):
        o = o.astype(np.float64).reshape(128, TPC * IC, 6)
        sq = (o[:, :, 2] + 256.0 * o[:, :, 1] ** 2
              + o[:, :, 5] + 256.0 * o[:, :, 4] ** 2)
        sq = sq.sum(axis=0).reshape(TPC, IC).sum(axis=1)
        for t, (P, Q) in enumerate(blocks_for_core(c)):
            if P < 8 and Q < 8:
                Bsum[0] += sq[t] * (1.0 if P == Q else 2.0)
            elif P >= 8 and Q >= 8:
                Bsum[1] += sq[t] * (1.0 if P == Q else 2.0)
            else:
                Bsum[2] += sq[t]

    cfg = [
        (hbS, hbS, Sq64, Sq64, sSq, sSq),   # xx
        (hbT, hbT, Tq64, Tq64, sTq, sTq),   # yy
        (hbS, hbT, Sq64, Tq64, sSq, sTq),   # xy: i-side S, j-side T
    ]
    c0 = np.exp(-2.0 / D)
    s = SCALE
    means = []
    for mat, (hb, hc, U, V, sU, sV) in enumerate(cfg):
        Sw = s * (sU @ sV) + N * hb.sum() + N * hc.sum()
        Sw2 = (s * s * Bsum[mat] + N * (hb ** 2).sum() + N * (hc ** 2).sum()
               + 2.0 * hb.sum() * hc.sum()
               + 2.0 * s * (hb @ (U @ sV) + hc @ (V @ sU)))
        means.append(c0 * (1.0 + (Sw + 0.5 * Sw2) / (float(N) * N)))
    return means


def kernel(source_features, target_features):
    S = np.asarray(source_features, dtype=np.float32)
    T = np.asarray(target_features, dtype=np.float32)

    nc = _get_nc()
    in_maps, Sb, Tb = _prep_inputs(S, T)
    import os
    trace = bool(int(os.environ.get("BASS_KERNEL_TRACE", "0")))
    res = bass_utils.run_bass_kernel_spmd(
        nc, in_maps, core_ids=list(range(NCORES)), trace=trace)
    _compiled["last_results"] = res
    per_core = [np.asarray(r["out"], np.float32) for r in res.results]

    means = _combine(per_core, S, T, Sb, Tb)
    f = np.float32
    xx, yy, xy = (f(m) for m in means)
    val = f(f(xx + yy) - f(2.0) * xy)
    return np.array(val, dtype=np.float32)
